# revision 20
# baseline (speedup 1.0000x reference)
"""AttentionInteractionNetwork GNN on 8 Trainium2 NeuronCores.

Sharding: edges partitioned by (relabeled) receiver so each core owns a
contiguous 1280-slot node chunk; receiver aggregation is core-local, the
sender aggregation partials are ReduceScattered. MLP matmuls run in
float32r (full PE rate); one-hot scatter matmuls run in bf16.
"""

import sys

sys.path.insert(0, "/opt/trn_rl_repo")

import numpy as np

import concourse.bass as bass
import concourse.bacc as bacc
import concourse.mybir as mybir
import concourse.tile as tile
from concourse.masks import make_identity
from concourse.bass_utils import run_bass_kernel_spmd

N_NODES = 10000
N_EDGES = 320000
D = 256
H = 512
EPS = 1e-5
N_CORES = 8
N_WIN = 80                 # 128-slot node windows
N_SLOTS = N_WIN * 128      # 10240 relabeled node slots
WIN_PER_CORE = N_WIN // N_CORES
CHUNK = WIN_PER_CORE * 128  # 1280 slots per core
DUMMY_SLOT = N_SLOTS - 1    # guaranteed dummy (10240 > 10000 real nodes)

f32 = mybir.dt.float32
f32r = mybir.dt.float32r
bf16 = mybir.dt.bfloat16
i32 = mybir.dt.int32

LAST_EXEC_NS = None
LAST_RESULTS = None
LAST_PP = None


# ----------------------------------------------------------------------------
# host-side preprocessing
# ----------------------------------------------------------------------------

def _relabel_nodes(receivers):
    """LPT-assign nodes to N_WIN windows of <=125 real nodes balancing
    in-degree. Returns slot_of_node [N_NODES] (node -> slot id)."""
    indeg = np.bincount(receivers, minlength=N_NODES)
    order = np.argsort(-indeg, kind="stable")
    win_load = np.zeros(N_WIN, dtype=np.int64)
    win_cnt = np.zeros(N_WIN, dtype=np.int64)
    win_members = [[] for _ in range(N_WIN)]
    # max real nodes per window so every window keeps some dummies is not
    # required; cap at 128.
    import heapq

    heap = [(0, 0, w) for w in range(N_WIN)]
    heapq.heapify(heap)
    for n in order:
        while True:
            load, cnt, w = heapq.heappop(heap)
            if win_cnt[w] < 128:
                break
        win_members[w].append(n)
        win_load[w] += indeg[n]
        win_cnt[w] += 1
        heapq.heappush(heap, (int(win_load[w]), int(win_cnt[w]), w))
    slot_of_node = np.full(N_NODES, -1, dtype=np.int64)
    for w in range(N_WIN):
        for j, n in enumerate(win_members[w]):
            slot_of_node[n] = w * 128 + j
    assert (slot_of_node >= 0).all()
    return slot_of_node


def _ceil_to(x, m):
    return ((x + m - 1) // m) * m


def _preprocess(nodes, edges, senders, receivers):
    senders = np.asarray(senders).astype(np.int64)
    receivers = np.asarray(receivers).astype(np.int64)
    slot_of_node = _relabel_nodes(receivers)
    r_slot = slot_of_node[receivers]   # [E]
    s_slot = slot_of_node[senders]     # [E]
    r_win = r_slot >> 7                # receiver window = core*10 + local
    core_of_edge = r_win // WIN_PER_CORE

    # --- pass-1 layout: per core, edges sorted by receiver window, each
    # window padded to TPW tiles of 128 ---
    per_cw_idx = [[None] * N_WIN for _ in range(N_CORES)]
    max_cnt = 0
    for c in range(N_CORES):
        em = np.nonzero(core_of_edge == c)[0]
        w_of = r_win[em]
        order = np.argsort(w_of, kind="stable")
        em = em[order]
        w_of = w_of[order]
        bounds = np.searchsorted(w_of, np.arange(c * WIN_PER_CORE, (c + 1) * WIN_PER_CORE + 1))
        for lw in range(WIN_PER_CORE):
            idx = em[bounds[lw]:bounds[lw + 1]]
            per_cw_idx[c][c * WIN_PER_CORE + lw] = idx
            max_cnt = max(max_cnt, len(idx))
    TPW = _ceil_to(max(_ceil_to(max_cnt, 128) // 128, 4), 4)  # tiles/window, mult of 4
    E_PAD = WIN_PER_CORE * TPW * 128

    # --- pass-2 layout: per core, edges grouped by sender window, each
    # group padded to TPW2 tiles of 128 ---
    max_cnt2 = 0
    per_c_sgroups = []
    for c in range(N_CORES):
        em = np.concatenate([per_cw_idx[c][c * WIN_PER_CORE + lw] for lw in range(WIN_PER_CORE)])
        sw = s_slot[em] >> 7
        order = np.argsort(sw, kind="stable")
        em2 = em[order]
        sw2 = sw[order]
        bounds = np.searchsorted(sw2, np.arange(N_WIN + 1))
        per_c_sgroups.append((em2, bounds))
        max_cnt2 = max(max_cnt2, int(np.max(bounds[1:] - bounds[:-1])))
    TPW2 = max(_ceil_to(max_cnt2, 128) // 128, 1)
    E_PAD2 = N_WIN * TPW2 * 128

    # --- per-core arrays ---
    pos_in_p2 = np.empty(N_EDGES, dtype=np.int64)  # global edge -> pass2 pos (per its core)
    cores = []
    nodes_rl = np.zeros((N_SLOTS, D), dtype=np.float32)
    nodes_rl[slot_of_node] = np.asarray(nodes, dtype=np.float32)
    for c in range(N_CORES):
        em2, bounds = per_c_sgroups[c]
        for w in range(N_WIN):
            seg = em2[bounds[w]:bounds[w + 1]]
            pos_in_p2[seg] = w * TPW2 * 128 + np.arange(len(seg))

        edge_ids = np.full(E_PAD, -1, dtype=np.int64)
        for lw in range(WIN_PER_CORE):
            idx = per_cw_idx[c][c * WIN_PER_CORE + lw]
            base = lw * TPW * 128
            edge_ids[base:base + len(idx)] = idx
        real = edge_ids >= 0
        eidx = edge_ids[real]

        e_feat = np.zeros((E_PAD, D), dtype=np.float32)
        e_feat[real] = np.asarray(edges, dtype=np.float32)[eidx]
        meta = np.empty((E_PAD, 4), dtype=np.int32)
        meta[:, 0] = DUMMY_SLOT; meta[:, 1] = DUMMY_SLOT
        meta[:, 2] = 1 << 30; meta[:, 3] = -1
        meta[real, 0] = s_slot[eidx].astype(np.int32)
        meta[real, 1] = r_slot[eidx].astype(np.int32)
        meta[real, 2] = pos_in_p2[eidx].astype(np.int32)
        meta[real, 3] = (r_slot[eidx] & 127).astype(np.int32)
        rslot = np.full(E_PAD, -1.0, dtype=np.float32)
        rslot[real] = (r_slot[eidx] & 127).astype(np.float32)
        sdest = np.full(E_PAD, 1 << 30, dtype=np.int32)
        sdest[real] = pos_in_p2[eidx].astype(np.int32)
        p2slot = np.full(E_PAD2, -1.0, dtype=np.float32)
        p2slot[pos_in_p2[eidx]] = (s_slot[eidx] & 127).astype(np.float32)

        cores.append(dict(
            edge_ids=edge_ids,
            edges_t=np.ascontiguousarray(e_feat.T),
            edges_r=e_feat,
            meta=meta, rslot=rslot, sdest=sdest, p2slot=p2slot,
            nodes_r_chunk=nodes_rl[c * CHUNK:(c + 1) * CHUNK],
            nodes_t_chunk=np.ascontiguousarray(nodes_rl[c * CHUNK:(c + 1) * CHUNK].T),
        ))
    return slot_of_node, nodes_rl, cores, TPW, E_PAD, TPW2, E_PAD2


# ----------------------------------------------------------------------------
# device program
# ----------------------------------------------------------------------------

def _mlp(nc, pools, xT_srcs, W_sb, b_sb, ktiles, width):
    """Feature-major 768->512->512->256 MLP. xT_srcs: list of 6 (tile, kslice)
    rhs sources [128, width]. Returns h3T sbuf tile [128, 2, width] (f32)."""
    sb, ps = pools
    W0, W1, W2 = W_sb
    b0, b1, b2 = b_sb
    h1 = sb.tile([128, 4, width], f32r, tag="h1")
    for m in range(4):
        p1 = ps.tile([128, width], f32, tag="ps_mlp")
        for k in range(6):
            src = xT_srcs[k]
            nc.tensor.matmul(p1[:], W0[:, k, m * 128:(m + 1) * 128], src,
                             start=(k == 0), stop=(k == 5))
        nc.scalar.activation(out=h1[:, m, :], in_=p1[:],
                             func=mybir.ActivationFunctionType.Silu,
                             bias=b0[:, m:m + 1], scale=1.0)
    h2 = sb.tile([128, 4, width], f32r, tag="h2")
    for m in range(4):
        p2 = ps.tile([128, width], f32, tag="ps_mlp")
        for k in range(4):
            nc.tensor.matmul(p2[:], W1[:, k, m * 128:(m + 1) * 128], h1[:, k, :],
                             start=(k == 0), stop=(k == 3))
        nc.scalar.activation(out=h2[:, m, :], in_=p2[:],
                             func=mybir.ActivationFunctionType.Silu,
                             bias=b1[:, m:m + 1], scale=1.0)
    h3 = sb.tile([128, 2, width], f32, tag="h3")
    for m in range(2):
        p3 = ps.tile([128, width], f32, tag="ps_mlp")
        for k in range(4):
            nc.tensor.matmul(p3[:], W2[:, k, m * 128:(m + 1) * 128], h2[:, k, :],
                             start=(k == 0), stop=(k == 3))
        nc.vector.tensor_scalar_add(out=h3[:, m, :], in0=p3[:], scalar1=b2[:, m:m + 1])
    return h3


def _ln_from_psum(nc, sb, u_ps, g_bc, b_bc, eps_t, width=D):
    """LayerNorm over free axis of u_ps [128, width] -> sbuf f32 tile."""
    stats = sb.tile([128, 6], f32, tag="ln_stats")
    nc.vector.bn_stats(out=stats[:], in_=u_ps[:])
    mv = sb.tile([128, 2], f32, tag="ln_mv")
    nc.vector.bn_aggr(out=mv[:], in_=stats[:])
    rstd = sb.tile([128, 1], f32, tag="ln_rstd")
    nc.scalar.activation(out=rstd[:], in_=mv[:, 1:2],
                         func=mybir.ActivationFunctionType.Sqrt,
                         bias=eps_t[:], scale=1.0)
    nc.vector.reciprocal(out=rstd[:], in_=rstd[:])
    u = sb.tile([128, width], f32, tag="ln_out")
    nc.vector.tensor_scalar(out=u[:], in0=u_ps[:], scalar1=mv[:, 0:1],
                            scalar2=rstd[:, :1],
                            op0=mybir.AluOpType.subtract, op1=mybir.AluOpType.mult)
    nc.vector.tensor_tensor(out=u[:], in0=u[:], in1=g_bc[:], op=mybir.AluOpType.mult)
    nc.vector.tensor_tensor(out=u[:], in0=u[:], in1=b_bc[:], op=mybir.AluOpType.add)
    return u


def build_program(TPW, E_PAD, TPW2, E_PAD2):
    nc = bacc.Bacc("TRN2", target_bir_lowering=False, debug=False, num_devices=N_CORES)
    A = mybir.ActivationFunctionType

    def din(name, shape, dt):
        return nc.dram_tensor(name, shape, dt, kind="ExternalInput").ap()

    edges_t = din("edges_t", [D, E_PAD], f32r)
    edges_r = din("edges_r", [E_PAD, D], f32)
    nodes_rl = din("nodes_rl", [N_SLOTS, D], f32)
    nodes_rc = din("nodes_rc", [CHUNK, D], f32)
    nodes_tc = din("nodes_tc", [D, CHUNK], f32r)
    meta_in = din("meta", [E_PAD, 4], i32)
    rslot_in = din("rslot", [E_PAD], f32)
    sdest_in = din("sdest", [E_PAD], i32)
    p2slot_in = din("p2slot", [E_PAD2], f32)
    wcat = din("wcat", [D, 2], f32r)
    attn_b = din("attn_b", [2], f32)
    eW0 = din("eW0", [3 * D, H], f32r); eW1 = din("eW1", [H, H], f32r); eW2 = din("eW2", [H, D], f32r)
    eb0 = din("eb0", [H], f32); eb1 = din("eb1", [H], f32); eb2 = din("eb2", [D], f32)
    eg = din("eg", [D], f32); ebt = din("ebt", [D], f32)
    nW0 = din("nW0", [3 * D, H], f32r); nW1 = din("nW1", [H, H], f32r); nW2 = din("nW2", [H, D], f32r)
    nb0 = din("nb0", [H], f32); nb1 = din("nb1", [H], f32); nb2 = din("nb2", [D], f32)
    ng = din("ng", [D], f32); nbt = din("nbt", [D], f32)

    out_edges = nc.dram_tensor("out_edges", [E_PAD, D], f32, kind="ExternalOutput").ap()
    out_nodes = nc.dram_tensor("out_nodes", [CHUNK, D], f32, kind="ExternalOutput").ap()
    dbg_ragg = nc.dram_tensor("dbg_ragg", [CHUNK, D], f32, kind="ExternalOutput").ap()
    dbg_sagg = nc.dram_tensor("dbg_sagg", [CHUNK, D], f32, kind="ExternalOutput").ap()

    T2 = N_WIN * TPW2

    with tile.TileContext(nc) as tc:
        consts = tc.alloc_tile_pool(name="consts", bufs=1)
        dram = tc.alloc_tile_pool(name="dram", bufs=1, space="DRAM")

        _uid = [0]

        def _tag(p):
            _uid[0] += 1
            return f"{p}{_uid[0]}"

        def load_w(ap, kt, mwid):
            t = consts.tile([128, kt, mwid], f32r, tag=_tag("w"))
            nc.sync.dma_start(out=t[:], in_=ap.rearrange("(k p) m -> p k m", p=128))
            return t

        def load_b(ap, mt):
            t = consts.tile([128, mt], f32, tag=_tag("b"))
            nc.sync.dma_start(out=t[:], in_=ap.rearrange("(m p) -> p m", p=128))
            return t

        def load_bc(ap, width):
            t = consts.tile([128, width], f32, tag=_tag("bc"))
            src = bass.AP(tensor=ap.tensor, offset=ap.offset, ap=[[0, 128]] + list(ap.ap))
            nc.gpsimd.dma_start(out=t[:], in_=src)
            return t

        eW = (load_w(eW0, 6, H), load_w(eW1, 4, H), load_w(eW2, 4, D))
        nW = (load_w(nW0, 6, H), load_w(nW1, 4, H), load_w(nW2, 4, D))
        eB = (load_b(eb0, 4), load_b(eb1, 4), load_b(eb2, 2))
        nB = (load_b(nb0, 4), load_b(nb1, 4), load_b(nb2, 2))
        eg_bc = load_bc(eg, D); ebt_bc = load_bc(ebt, D)
        ng_bc = load_bc(ng, D); nbt_bc = load_bc(nbt, D)
        wcat_sb = consts.tile([128, 2, 2], f32r)
        nc.sync.dma_start(out=wcat_sb[:], in_=wcat.rearrange("(k p) t -> p k t", p=128))
        ab_bc = load_bc(attn_b, 2)  # [128, 2]: col0 = recv bias, col1 = send bias
        iota_i = consts.tile([128, 128], i32)
        nc.gpsimd.iota(iota_i[:], pattern=[[1, 128]], base=0, channel_multiplier=0)
        iota_f = consts.tile([128, 128], f32)
        nc.vector.tensor_copy(out=iota_f[:], in_=iota_i[:])
        ident = consts.tile([128, 128], f32)
        make_identity(nc, ident[:])
        recvT = consts.tile([128, 2, CHUNK], f32r)   # recv_agg feature-major
        sentT = consts.tile([128, 2, CHUNK], f32r)   # sent_agg feature-major
        eps30 = consts.tile([128, 1], f32)
        nc.vector.memset(eps30[:], 1e-30)
        eps_ln = consts.tile([128, 1], f32)
        nc.vector.memset(eps_ln[:], EPS)

        scratch = dram.tile([E_PAD2, 258], bf16)
        send_part = dram.tile([N_SLOTS, 257], f32)
        rs_out = dram.tile([CHUNK, 257], f32)

        # ---- memset pass-2 scratch ----
        with tc.tile_pool(name="ms", bufs=2) as ms:
            z = ms.tile([128, 8, 258], bf16)
            nc.vector.memset(z[:], 0.0)
            sview = scratch[:].rearrange("(t p) c -> p t c", p=128)
            for i in range(0, T2, 8):
                nc.sync.dma_start(out=sview[:, i:i + 8, :], in_=z[:])

        # ================= PASS 1: edges =================
        with tc.tile_pool(name="p1sb", bufs=2) as sb, \
             tc.tile_pool(name="p1ps", bufs=2, space="PSUM") as ps, \
             tc.tile_pool(name="p1win", bufs=1, space="PSUM") as psw:
            edges_t_v = edges_t.rearrange("(k p) e -> p k e", p=128)
            n_super = TPW // 4
            for lw in range(WIN_PER_CORE):
                ps_win = psw.tile([128, 257], f32, tag="win")
                for sup in range(n_super):
                    st0 = (lw * TPW + sup * 4) * 128  # first edge of super-tile
                    xT = sb.tile([128, 6, 512], f32r, tag="xT")
                    nc.sync.dma_start(out=xT[:, 0:2, :], in_=edges_t_v[:, :, st0:st0 + 512])
                    sub_data = []
                    for s in range(4):
                        e0 = st0 + s * 128
                        gi = sb.tile([128, 4], i32, tag="gi", bufs=6)
                        nc.sync.dma_start(out=gi[:], in_=meta_in[e0:e0 + 128, :])
                        for gcol, koff in ((0, 2), (1, 4)):
                            g = sb.tile([128, D], f32, tag="gath", bufs=4)
                            nc.gpsimd.indirect_dma_start(
                                out=g[:], out_offset=None, in_=nodes_rl[:],
                                in_offset=bass.IndirectOffsetOnAxis(ap=gi[:, gcol:gcol + 1], axis=0))
                            for k in range(2):
                                tp = ps.tile([128, 128], f32, tag="ps_tr")
                                nc.tensor.transpose(out=tp[:], in_=g[:, k * 128:(k + 1) * 128],
                                                    identity=ident[:])
                                nc.vector.tensor_copy(out=xT[:, koff + k, s * 128:(s + 1) * 128], in_=tp[:])
                        # logits -> exp
                        plg = ps.tile([128, 2], f32, tag="ps_lg", bufs=1)
                        for k in range(2):
                            nc.tensor.matmul(plg[:], xT[:, k, s * 128:(s + 1) * 128],
                                             wcat_sb[:, k, :], start=(k == 0), stop=(k == 1))
                        exps = sb.tile([128, 2], f32, tag="exps", bufs=6)
                        nc.scalar.activation(out=exps[:, 0:1], in_=plg[:, 0:1], func=A.Exp,
                                             bias=ab_bc[:, 0:1], scale=1.0)
                        nc.scalar.activation(out=exps[:, 1:2], in_=plg[:, 1:2], func=A.Exp,
                                             bias=ab_bc[:, 1:2], scale=1.0)
                        sub_data.append((e0, exps))
                    h3 = _mlp(nc, (sb, ps), [xT[:, k, :] for k in range(6)], eW, eB, 6, 512)
                    for s in range(4):
                        e0, exps = sub_data[s]
                        ups = ps.tile([128, D], f32, tag="ps_ups", bufs=1)
                        for k in range(2):
                            tp2 = ps.tile([128, 128], f32, tag="ps_tr")
                            nc.tensor.transpose(out=tp2[:], in_=h3[:, k, s * 128:(s + 1) * 128],
                                                identity=ident[:])
                            nc.vector.tensor_copy(out=ups[:, k * 128:(k + 1) * 128], in_=tp2[:])
                        u = _ln_from_psum(nc, sb, ups, eg_bc, ebt_bc, eps_ln)
                        # residual edge output
                        er = sb.tile([128, D], f32, tag="er", bufs=4)
                        nc.sync.dma_start(out=er[:], in_=edges_r[e0:e0 + 128, :])
                        oe = sb.tile([128, D], f32, tag="oe", bufs=4)
                        nc.vector.tensor_tensor(out=oe[:], in0=u[:], in1=er[:], op=mybir.AluOpType.add)
                        nc.scalar.dma_start(out=out_edges[e0:e0 + 128, :], in_=oe[:])
                        # pack [U | 1 | exp_s] bf16 and scatter to pass-2 scratch
                        ub = sb.tile([128, 258], bf16, tag="ub", bufs=4)
                        nc.vector.tensor_copy(out=ub[:, 0:256], in_=u[:])
                        nc.vector.memset(ub[:, 256:257], 1.0)
                        nc.vector.tensor_copy(out=ub[:, 257:258], in_=exps[:, 1:2])
                        sd = sb.tile([128, 1], i32, tag="sd", bufs=6)
                        nc.sync.dma_start(out=sd[:, 0:1], in_=sdest_in[e0:e0 + 128, None])
                        nc.gpsimd.indirect_dma_start(
                            out=scratch[:], out_offset=bass.IndirectOffsetOnAxis(ap=sd[:, :1], axis=0),
                            in_=ub[:], in_offset=None,
                            bounds_check=E_PAD2 - 1, oob_is_err=False)
                        # receiver one-hot aggregation
                        rs = sb.tile([128, 1], f32, tag="rs", bufs=6)
                        nc.sync.dma_start(out=rs[:, 0:1], in_=rslot_in[e0:e0 + 128, None])
                        oh = sb.tile([128, 128], bf16, tag="oh", bufs=4)
                        nc.vector.tensor_tensor(out=oh[:], in0=rs[:, :1].to_broadcast([128, 128]),
                                                in1=iota_f[:], op=mybir.AluOpType.is_equal)
                        S = sb.tile([128, 128], bf16, tag="S", bufs=4)
                        nc.vector.tensor_scalar_mul(out=S[:], in0=oh[:], scalar1=exps[:, 0:1])
                        first = (sup == 0 and s == 0)
                        last = (sup == n_super - 1 and s == 3)
                        nc.tensor.matmul(ps_win[:], S[:], ub[:, 0:257], start=first, stop=last)
                # window close: normalize and transpose into recvT
                den = sb.tile([128, 1], f32, tag="den")
                nc.vector.tensor_tensor(out=den[:], in0=ps_win[:, 256:257], in1=eps30[:],
                                        op=mybir.AluOpType.add)
                nc.vector.reciprocal(out=den[:], in_=den[:])
                agg = sb.tile([128, D], f32, tag="agg")
                nc.vector.tensor_scalar_mul(out=agg[:], in0=ps_win[:, 0:256], scalar1=den[:, :1])
                nc.scalar.dma_start(out=dbg_ragg[lw * 128:(lw + 1) * 128, :], in_=agg[:])
                for k in range(2):
                    tp3 = ps.tile([128, 128], f32, tag="ps_tr")
                    nc.tensor.transpose(out=tp3[:], in_=agg[:, k * 128:(k + 1) * 128], identity=ident[:])
                    nc.vector.tensor_copy(out=recvT[:, k, lw * 128:(lw + 1) * 128], in_=tp3[:])

        # ================= PASS 2: sender aggregation =================
        with tc.tile_pool(name="p2sb", bufs=3) as sb, \
             tc.tile_pool(name="p2ps", bufs=2, space="PSUM") as ps:
            sc_v = scratch[:].rearrange("(t p) c -> p t c", p=128)
            for w in range(N_WIN):
                pw = ps.tile([128, 257], f32, tag="p2win")
                for i in range(TPW2):
                    t2 = w * TPW2 + i
                    sct = sb.tile([128, 258], bf16, tag="sct")
                    nc.sync.dma_start(out=sct[:], in_=sc_v[:, t2, :])
                    sl = sb.tile([128, 1], f32, tag="sl")
                    nc.sync.dma_start(out=sl[:, 0:1], in_=p2slot_in[t2 * 128:(t2 + 1) * 128, None])
                    oh2 = sb.tile([128, 128], bf16, tag="oh2")
                    nc.vector.tensor_tensor(out=oh2[:], in0=sl[:, :1].to_broadcast([128, 128]),
                                            in1=iota_f[:], op=mybir.AluOpType.is_equal)
                    exf = sb.tile([128, 1], f32, tag="exf")
                    nc.vector.tensor_copy(out=exf[:], in_=sct[:, 257:258])
                    S2 = sb.tile([128, 128], bf16, tag="S2")
                    nc.vector.tensor_scalar_mul(out=S2[:], in0=oh2[:], scalar1=exf[:, :1])
                    nc.tensor.matmul(pw[:], S2[:], sct[:, 0:257], start=(i == 0), stop=(i == TPW2 - 1))
                po = sb.tile([128, 257], f32, tag="po")
                nc.vector.tensor_copy(out=po[:], in_=pw[:])
                nc.sync.dma_start(out=send_part[w * 128:(w + 1) * 128, :], in_=po[:])

        # ================= ReduceScatter + node MLP =================
        nc.gpsimd.collective_compute(
            "ReduceScatter", mybir.AluOpType.add,
            replica_groups=[list(range(N_CORES))],
            ins=[send_part.opt()], outs=[rs_out.opt()])

        with tc.tile_pool(name="p3sb", bufs=2) as sb, \
             tc.tile_pool(name="p3ps", bufs=2, space="PSUM") as ps:
            for nw in range(WIN_PER_CORE):
                rst = sb.tile([128, 257], f32, tag="rst")
                nc.sync.dma_start(out=rst[:], in_=rs_out[nw * 128:(nw + 1) * 128, :])
                den = sb.tile([128, 1], f32, tag="den3")
                nc.vector.tensor_tensor(out=den[:], in0=rst[:, 256:257], in1=eps30[:],
                                        op=mybir.AluOpType.add)
                nc.vector.reciprocal(out=den[:], in_=den[:])
                sagg = sb.tile([128, D], f32, tag="sagg")
                nc.vector.tensor_scalar_mul(out=sagg[:], in0=rst[:, 0:256], scalar1=den[:, :1])
                nc.scalar.dma_start(out=dbg_sagg[nw * 128:(nw + 1) * 128, :], in_=sagg[:])
                for k in range(2):
                    tp = ps.tile([128, 128], f32, tag="ps_tr3")
                    nc.tensor.transpose(out=tp[:], in_=sagg[:, k * 128:(k + 1) * 128], identity=ident[:])
                    nc.vector.tensor_copy(out=sentT[:, k, nw * 128:(nw + 1) * 128], in_=tp[:])
            nodes_t_v = nodes_tc.rearrange("(k p) e -> p k e", p=128)
            for c0 in range(0, CHUNK, 512):
                wid = min(512, CHUNK - c0)
                nT = sb.tile([128, 2, wid], f32r, tag="nT")
                nc.sync.dma_start(out=nT[:], in_=nodes_t_v[:, :, c0:c0 + wid])
                srcs = ([nT[:, k, :] for k in range(2)]
                        + [recvT[:, k, c0:c0 + wid] for k in range(2)]
                        + [sentT[:, k, c0:c0 + wid] for k in range(2)])
                h3 = _mlp(nc, (sb, ps), srcs, nW, nB, 6, wid)
                for s in range(wid // 128):
                    n0 = c0 + s * 128
                    ups = ps.tile([128, D], f32, tag="ps_ups3")
                    for k in range(2):
                        tp2 = ps.tile([128, 128], f32, tag="ps_tr3")
                        nc.tensor.transpose(out=tp2[:], in_=h3[:, k, s * 128:(s + 1) * 128],
                                            identity=ident[:])
                        nc.vector.tensor_copy(out=ups[:, k * 128:(k + 1) * 128], in_=tp2[:])
                    un = _ln_from_psum(nc, sb, ups, ng_bc, nbt_bc, eps_ln)
                    nr = sb.tile([128, D], f32, tag="nr")
                    nc.sync.dma_start(out=nr[:], in_=nodes_rc[n0:n0 + 128, :])
                    on = sb.tile([128, D], f32, tag="on")
                    nc.vector.tensor_tensor(out=on[:], in0=un[:], in1=nr[:], op=mybir.AluOpType.add)
                    nc.scalar.dma_start(out=out_nodes[n0:n0 + 128, :], in_=on[:])

        consts.release()
        dram.release()

    nc.compile()
    return nc


# ----------------------------------------------------------------------------
# entry point
# ----------------------------------------------------------------------------

def kernel(nodes, edges, senders, receivers,
           edge_W0, edge_b0, edge_W1, edge_b1, edge_W2, edge_b2, edge_ln_g, edge_ln_b,
           node_W0, node_b0, node_W1, node_b1, node_W2, node_b2, node_ln_g, node_ln_b,
           recv_attn_w, recv_attn_b, send_attn_w, send_attn_b):
    global LAST_EXEC_NS
    import os
    nodes = np.asarray(nodes, dtype=np.float32)
    edges = np.asarray(edges, dtype=np.float32)
    slot_of_node, nodes_rl, cores, TPW, E_PAD, TPW2, E_PAD2 = _preprocess(
        nodes, edges, senders, receivers)
    nc = build_program(TPW, E_PAD, TPW2, E_PAD2)

    wcat = np.concatenate([np.asarray(recv_attn_w), np.asarray(send_attn_w)], axis=1).astype(np.float32)
    attn_b = np.concatenate([np.asarray(recv_attn_b), np.asarray(send_attn_b)]).astype(np.float32)
    shared = dict(
        nodes_rl=nodes_rl, wcat=wcat, attn_b=attn_b,
        eW0=np.asarray(edge_W0, np.float32), eW1=np.asarray(edge_W1, np.float32),
        eW2=np.asarray(edge_W2, np.float32),
        eb0=np.asarray(edge_b0, np.float32), eb1=np.asarray(edge_b1, np.float32),
        eb2=np.asarray(edge_b2, np.float32),
        eg=np.asarray(edge_ln_g, np.float32), ebt=np.asarray(edge_ln_b, np.float32),
        nW0=np.asarray(node_W0, np.float32), nW1=np.asarray(node_W1, np.float32),
        nW2=np.asarray(node_W2, np.float32),
        nb0=np.asarray(node_b0, np.float32), nb1=np.asarray(node_b1, np.float32),
        nb2=np.asarray(node_b2, np.float32),
        ng=np.asarray(node_ln_g, np.float32), nbt=np.asarray(node_ln_b, np.float32),
    )
    in_maps = []
    for c in range(N_CORES):
        m = dict(shared)
        for k in ("edges_t", "edges_r", "meta", "rslot", "sdest", "p2slot",
                  "nodes_r_chunk", "nodes_t_chunk"):
            tgt = {"nodes_r_chunk": "nodes_rc", "nodes_t_chunk": "nodes_tc"}.get(k, k)
            m[tgt] = cores[c][k]
        in_maps.append(m)

    trace = bool(int(os.environ.get("KERNEL_TRACE", "0")))
    res = run_bass_kernel_spmd(nc, in_maps, list(range(N_CORES)), trace=trace)
    LAST_EXEC_NS = res.exec_time_ns
    global LAST_RESULTS, LAST_PP
    LAST_RESULTS = res.results
    LAST_PP = (slot_of_node, cores)

    edges_out = np.empty((N_EDGES, D), dtype=np.float32)
    nodes_out = np.empty((N_NODES, D), dtype=np.float32)
    inv_slot = np.full(N_SLOTS, -1, dtype=np.int64)
    inv_slot[slot_of_node] = np.arange(N_NODES)
    for c in range(N_CORES):
        eo = res.results[c]["out_edges"]
        ids = cores[c]["edge_ids"]
        real = ids >= 0
        edges_out[ids[real]] = eo[real]
        no = res.results[c]["out_nodes"]
        sl = inv_slot[c * CHUNK:(c + 1) * CHUNK]
        rm = sl >= 0
        nodes_out[sl[rm]] = no[rm]
    return nodes_out, edges_out


# revision 21
# speedup vs baseline: 1.0450x; 1.0450x over previous
"""AttentionInteractionNetwork GNN on 8 Trainium2 NeuronCores.

Sharding: edges partitioned by (relabeled) receiver so each core owns a
contiguous 1280-slot node chunk; receiver aggregation is core-local, the
sender aggregation partials are ReduceScattered. MLP matmuls run in
float32r (full PE rate); one-hot scatter matmuls run in bf16.
"""

import sys

sys.path.insert(0, "/opt/trn_rl_repo")

import numpy as np

import concourse.bass as bass
import concourse.bacc as bacc
import concourse.mybir as mybir
import concourse.tile as tile
from concourse.masks import make_identity
from concourse.bass_utils import run_bass_kernel_spmd

N_NODES = 10000
N_EDGES = 320000
D = 256
H = 512
EPS = 1e-5
N_CORES = 8
N_WIN = 80                 # 128-slot node windows
N_SLOTS = N_WIN * 128      # 10240 relabeled node slots
WIN_PER_CORE = N_WIN // N_CORES
CHUNK = WIN_PER_CORE * 128  # 1280 slots per core
DUMMY_SLOT = N_SLOTS - 1    # guaranteed dummy (10240 > 10000 real nodes)

f32 = mybir.dt.float32
f32r = mybir.dt.float32r
bf16 = mybir.dt.bfloat16
i32 = mybir.dt.int32

LAST_EXEC_NS = None
LAST_RESULTS = None
LAST_PP = None


# ----------------------------------------------------------------------------
# host-side preprocessing
# ----------------------------------------------------------------------------

def _relabel_nodes(receivers):
    """LPT-assign nodes to N_WIN windows of <=125 real nodes balancing
    in-degree. Returns slot_of_node [N_NODES] (node -> slot id)."""
    indeg = np.bincount(receivers, minlength=N_NODES)
    order = np.argsort(-indeg, kind="stable")
    win_load = np.zeros(N_WIN, dtype=np.int64)
    win_cnt = np.zeros(N_WIN, dtype=np.int64)
    win_members = [[] for _ in range(N_WIN)]
    # max real nodes per window so every window keeps some dummies is not
    # required; cap at 128.
    import heapq

    heap = [(0, 0, w) for w in range(N_WIN)]
    heapq.heapify(heap)
    for n in order:
        while True:
            load, cnt, w = heapq.heappop(heap)
            if win_cnt[w] < 128:
                break
        win_members[w].append(n)
        win_load[w] += indeg[n]
        win_cnt[w] += 1
        heapq.heappush(heap, (int(win_load[w]), int(win_cnt[w]), w))
    slot_of_node = np.full(N_NODES, -1, dtype=np.int64)
    for w in range(N_WIN):
        for j, n in enumerate(win_members[w]):
            slot_of_node[n] = w * 128 + j
    assert (slot_of_node >= 0).all()
    return slot_of_node


def _ceil_to(x, m):
    return ((x + m - 1) // m) * m


def _preprocess(nodes, edges, senders, receivers):
    senders = np.asarray(senders).astype(np.int64)
    receivers = np.asarray(receivers).astype(np.int64)
    slot_of_node = _relabel_nodes(receivers)
    r_slot = slot_of_node[receivers]   # [E]
    s_slot = slot_of_node[senders]     # [E]
    r_win = r_slot >> 7                # receiver window = core*10 + local
    core_of_edge = r_win // WIN_PER_CORE

    # --- pass-1 layout: per core, edges sorted by receiver window, each
    # window padded to TPW tiles of 128 ---
    per_cw_idx = [[None] * N_WIN for _ in range(N_CORES)]
    max_cnt = 0
    for c in range(N_CORES):
        em = np.nonzero(core_of_edge == c)[0]
        w_of = r_win[em]
        order = np.argsort(w_of, kind="stable")
        em = em[order]
        w_of = w_of[order]
        bounds = np.searchsorted(w_of, np.arange(c * WIN_PER_CORE, (c + 1) * WIN_PER_CORE + 1))
        for lw in range(WIN_PER_CORE):
            idx = em[bounds[lw]:bounds[lw + 1]]
            per_cw_idx[c][c * WIN_PER_CORE + lw] = idx
            max_cnt = max(max_cnt, len(idx))
    TPW = _ceil_to(max(_ceil_to(max_cnt, 128) // 128, 4), 4)  # tiles/window, mult of 4
    E_PAD = WIN_PER_CORE * TPW * 128

    # --- pass-2 layout: per core, edges grouped by sender window, each
    # group padded to TPW2 tiles of 128 ---
    max_cnt2 = 0
    per_c_sgroups = []
    for c in range(N_CORES):
        em = np.concatenate([per_cw_idx[c][c * WIN_PER_CORE + lw] for lw in range(WIN_PER_CORE)])
        sw = s_slot[em] >> 7
        order = np.argsort(sw, kind="stable")
        em2 = em[order]
        sw2 = sw[order]
        bounds = np.searchsorted(sw2, np.arange(N_WIN + 1))
        per_c_sgroups.append((em2, bounds))
        max_cnt2 = max(max_cnt2, int(np.max(bounds[1:] - bounds[:-1])))
    TPW2 = max(_ceil_to(max_cnt2, 128) // 128, 1)
    E_PAD2 = N_WIN * TPW2 * 128

    # --- per-core arrays ---
    pos_in_p2 = np.empty(N_EDGES, dtype=np.int64)  # global edge -> pass2 pos (per its core)
    cores = []
    nodes_rl = np.zeros((N_SLOTS, D), dtype=np.float32)
    nodes_rl[slot_of_node] = np.asarray(nodes, dtype=np.float32)
    for c in range(N_CORES):
        em2, bounds = per_c_sgroups[c]
        for w in range(N_WIN):
            seg = em2[bounds[w]:bounds[w + 1]]
            pos_in_p2[seg] = w * TPW2 * 128 + np.arange(len(seg))

        edge_ids = np.full(E_PAD, -1, dtype=np.int64)
        for lw in range(WIN_PER_CORE):
            idx = per_cw_idx[c][c * WIN_PER_CORE + lw]
            base = lw * TPW * 128
            edge_ids[base:base + len(idx)] = idx
        real = edge_ids >= 0
        eidx = edge_ids[real]

        e_feat = np.zeros((E_PAD, D), dtype=np.float32)
        e_feat[real] = np.asarray(edges, dtype=np.float32)[eidx]
        meta = np.empty((E_PAD, 4), dtype=np.int32)
        meta[:, 0] = DUMMY_SLOT; meta[:, 1] = DUMMY_SLOT
        meta[:, 2] = 1 << 30; meta[:, 3] = -1
        meta[real, 0] = s_slot[eidx].astype(np.int32)
        meta[real, 1] = r_slot[eidx].astype(np.int32)
        meta[real, 2] = pos_in_p2[eidx].astype(np.int32)
        meta[real, 3] = (r_slot[eidx] & 127).astype(np.int32)
        rslot = np.full(E_PAD, -1.0, dtype=np.float32)
        rslot[real] = (r_slot[eidx] & 127).astype(np.float32)
        sdest = np.full(E_PAD, 1 << 30, dtype=np.int32)
        sdest[real] = pos_in_p2[eidx].astype(np.int32)
        p2slot = np.full(E_PAD2, -1.0, dtype=np.float32)
        p2slot[pos_in_p2[eidx]] = (s_slot[eidx] & 127).astype(np.float32)

        cores.append(dict(
            edge_ids=edge_ids,
            edges_t=np.ascontiguousarray(e_feat.T),
            edges_r=e_feat,
            meta=meta, rslot=rslot, sdest=sdest, p2slot=p2slot,
            nodes_r_chunk=nodes_rl[c * CHUNK:(c + 1) * CHUNK],
            nodes_t_chunk=np.ascontiguousarray(nodes_rl[c * CHUNK:(c + 1) * CHUNK].T),
        ))
    return slot_of_node, nodes_rl, cores, TPW, E_PAD, TPW2, E_PAD2


# ----------------------------------------------------------------------------
# device program
# ----------------------------------------------------------------------------

def _mlp(nc, pools, xT_srcs, W_sb, b_sb, ktiles, width):
    """Feature-major 768->512->512->256 MLP. xT_srcs: list of 6 (tile, kslice)
    rhs sources [128, width]. Returns h3T sbuf tile [128, 2, width] (f32)."""
    sb, ps = pools
    W0, W1, W2 = W_sb
    b0, b1, b2 = b_sb
    h1 = sb.tile([128, 4, width], f32r, tag="h1")
    for m in range(4):
        p1 = ps.tile([128, width], f32, tag="ps_mlp")
        for k in range(6):
            src = xT_srcs[k]
            nc.tensor.matmul(p1[:], W0[:, k, m * 128:(m + 1) * 128], src,
                             start=(k == 0), stop=(k == 5))
        nc.scalar.activation(out=h1[:, m, :], in_=p1[:],
                             func=mybir.ActivationFunctionType.Silu,
                             bias=b0[:, m:m + 1], scale=1.0)
    h2 = sb.tile([128, 4, width], f32r, tag="h2")
    for m in range(4):
        p2 = ps.tile([128, width], f32, tag="ps_mlp")
        for k in range(4):
            nc.tensor.matmul(p2[:], W1[:, k, m * 128:(m + 1) * 128], h1[:, k, :],
                             start=(k == 0), stop=(k == 3))
        nc.scalar.activation(out=h2[:, m, :], in_=p2[:],
                             func=mybir.ActivationFunctionType.Silu,
                             bias=b1[:, m:m + 1], scale=1.0)
    h3 = sb.tile([128, 2, width], f32, tag="h3")
    for m in range(2):
        p3 = ps.tile([128, width], f32, tag="ps_mlp")
        for k in range(4):
            nc.tensor.matmul(p3[:], W2[:, k, m * 128:(m + 1) * 128], h2[:, k, :],
                             start=(k == 0), stop=(k == 3))
        nc.vector.tensor_scalar_add(out=h3[:, m, :], in0=p3[:], scalar1=b2[:, m:m + 1])
    return h3


def _ln_from_psum(nc, sb, u_ps, g_bc, b_bc, eps_t, width=D):
    """LayerNorm over free axis of u_ps [128, width] -> sbuf f32 tile."""
    stats = sb.tile([128, 6], f32, tag="ln_stats")
    nc.vector.bn_stats(out=stats[:], in_=u_ps[:])
    mv = sb.tile([128, 2], f32, tag="ln_mv")
    nc.vector.bn_aggr(out=mv[:], in_=stats[:])
    rstd = sb.tile([128, 1], f32, tag="ln_rstd")
    nc.scalar.activation(out=rstd[:], in_=mv[:, 1:2],
                         func=mybir.ActivationFunctionType.Sqrt,
                         bias=eps_t[:], scale=1.0)
    nc.vector.reciprocal(out=rstd[:], in_=rstd[:])
    u = sb.tile([128, width], f32, tag="ln_out")
    nc.vector.tensor_scalar(out=u[:], in0=u_ps[:], scalar1=mv[:, 0:1],
                            scalar2=rstd[:, :1],
                            op0=mybir.AluOpType.subtract, op1=mybir.AluOpType.mult)
    nc.vector.tensor_tensor(out=u[:], in0=u[:], in1=g_bc[:], op=mybir.AluOpType.mult)
    nc.vector.tensor_tensor(out=u[:], in0=u[:], in1=b_bc[:], op=mybir.AluOpType.add)
    return u


def build_program(TPW, E_PAD, TPW2, E_PAD2):
    nc = bacc.Bacc("TRN2", target_bir_lowering=False, debug=False, num_devices=N_CORES)
    A = mybir.ActivationFunctionType

    def din(name, shape, dt):
        return nc.dram_tensor(name, shape, dt, kind="ExternalInput").ap()

    edges_t = din("edges_t", [D, E_PAD], f32r)
    edges_r = din("edges_r", [E_PAD, D], f32)
    nodes_rl = din("nodes_rl", [N_SLOTS, D], f32)
    nodes_rc = din("nodes_rc", [CHUNK, D], f32)
    nodes_tc = din("nodes_tc", [D, CHUNK], f32r)
    meta_in = din("meta", [E_PAD, 4], i32)
    rslot_in = din("rslot", [E_PAD], f32)
    sdest_in = din("sdest", [E_PAD], i32)
    p2slot_in = din("p2slot", [E_PAD2], f32)
    wcat = din("wcat", [D, 2], f32r)
    attn_b = din("attn_b", [2], f32)
    eW0 = din("eW0", [3 * D, H], f32r); eW1 = din("eW1", [H, H], f32r); eW2 = din("eW2", [H, D], f32r)
    eb0 = din("eb0", [H], f32); eb1 = din("eb1", [H], f32); eb2 = din("eb2", [D], f32)
    eg = din("eg", [D], f32); ebt = din("ebt", [D], f32)
    nW0 = din("nW0", [3 * D, H], f32r); nW1 = din("nW1", [H, H], f32r); nW2 = din("nW2", [H, D], f32r)
    nb0 = din("nb0", [H], f32); nb1 = din("nb1", [H], f32); nb2 = din("nb2", [D], f32)
    ng = din("ng", [D], f32); nbt = din("nbt", [D], f32)

    out_edges = nc.dram_tensor("out_edges", [E_PAD, D], f32, kind="ExternalOutput").ap()
    out_nodes = nc.dram_tensor("out_nodes", [CHUNK, D], f32, kind="ExternalOutput").ap()
    dbg_ragg = nc.dram_tensor("dbg_ragg", [CHUNK, D], f32, kind="ExternalOutput").ap()
    dbg_sagg = nc.dram_tensor("dbg_sagg", [CHUNK, D], f32, kind="ExternalOutput").ap()

    T2 = N_WIN * TPW2

    with tile.TileContext(nc) as tc:
        consts = tc.alloc_tile_pool(name="consts", bufs=1)
        dram = tc.alloc_tile_pool(name="dram", bufs=1, space="DRAM")

        _uid = [0]

        def _tag(p):
            _uid[0] += 1
            return f"{p}{_uid[0]}"

        def load_w(ap, kt, mwid):
            t = consts.tile([128, kt, mwid], f32r, tag=_tag("w"))
            nc.sync.dma_start(out=t[:], in_=ap.rearrange("(k p) m -> p k m", p=128))
            return t

        def load_b(ap, mt):
            t = consts.tile([128, mt], f32, tag=_tag("b"))
            nc.sync.dma_start(out=t[:], in_=ap.rearrange("(m p) -> p m", p=128))
            return t

        def load_bc(ap, width):
            t = consts.tile([128, width], f32, tag=_tag("bc"))
            src = bass.AP(tensor=ap.tensor, offset=ap.offset, ap=[[0, 128]] + list(ap.ap))
            nc.gpsimd.dma_start(out=t[:], in_=src)
            return t

        eW = (load_w(eW0, 6, H), load_w(eW1, 4, H), load_w(eW2, 4, D))
        nW = (load_w(nW0, 6, H), load_w(nW1, 4, H), load_w(nW2, 4, D))
        eB = (load_b(eb0, 4), load_b(eb1, 4), load_b(eb2, 2))
        nB = (load_b(nb0, 4), load_b(nb1, 4), load_b(nb2, 2))
        eg_bc = load_bc(eg, D); ebt_bc = load_bc(ebt, D)
        ng_bc = load_bc(ng, D); nbt_bc = load_bc(nbt, D)
        wcat_sb = consts.tile([128, 2, 2], f32r)
        nc.sync.dma_start(out=wcat_sb[:], in_=wcat.rearrange("(k p) t -> p k t", p=128))
        ab_bc = load_bc(attn_b, 2)  # [128, 2]: col0 = recv bias, col1 = send bias
        iota_i = consts.tile([128, 128], i32)
        nc.gpsimd.iota(iota_i[:], pattern=[[1, 128]], base=0, channel_multiplier=0)
        iota_f = consts.tile([128, 128], f32)
        nc.vector.tensor_copy(out=iota_f[:], in_=iota_i[:])
        ident = consts.tile([128, 128], f32)
        make_identity(nc, ident[:])
        recvT = consts.tile([128, 2, CHUNK], f32r)   # recv_agg feature-major
        sentT = consts.tile([128, 2, CHUNK], f32r)   # sent_agg feature-major
        eps30 = consts.tile([128, 1], f32)
        nc.vector.memset(eps30[:], 1e-30)
        eps_ln = consts.tile([128, 1], f32)
        nc.vector.memset(eps_ln[:], EPS)

        scratch = dram.tile([E_PAD2, 258], bf16)
        send_part = dram.tile([N_SLOTS, 257], f32)
        rs_out = dram.tile([CHUNK, 257], f32)

        # ---- memset pass-2 scratch ----
        with tc.tile_pool(name="ms", bufs=2) as ms:
            z = ms.tile([128, 8, 258], bf16)
            nc.vector.memset(z[:], 0.0)
            sview = scratch[:].rearrange("(t p) c -> p t c", p=128)
            for i in range(0, T2, 8):
                nc.sync.dma_start(out=sview[:, i:i + 8, :], in_=z[:])

        # ================= PASS 1: edges =================
        with tc.tile_pool(name="p1sb", bufs=2) as sb, \
             tc.tile_pool(name="p1ps", bufs=2, space="PSUM") as ps, \
             tc.tile_pool(name="p1win", bufs=1, space="PSUM") as psw:
            edges_t_v = edges_t.rearrange("(k p) e -> p k e", p=128)
            n_super = TPW // 4
            for lw in range(WIN_PER_CORE):
                ps_win = psw.tile([128, 257], f32, tag="win")
                for sup in range(n_super):
                    st0 = (lw * TPW + sup * 4) * 128  # first edge of super-tile
                    xT = sb.tile([128, 6, 512], f32r, tag="xT", bufs=3)
                    nc.sync.dma_start(out=xT[:, 0:2, :], in_=edges_t_v[:, :, st0:st0 + 512])
                    sub_data = []
                    for s in range(4):
                        e0 = st0 + s * 128
                        gi = sb.tile([128, 4], i32, tag="gi", bufs=12)
                        nc.sync.dma_start(out=gi[:], in_=meta_in[e0:e0 + 128, :])
                        for gcol, koff in ((0, 2), (1, 4)):
                            g = sb.tile([128, D], f32, tag="gath", bufs=10)
                            nc.gpsimd.indirect_dma_start(
                                out=g[:], out_offset=None, in_=nodes_rl[:],
                                in_offset=bass.IndirectOffsetOnAxis(ap=gi[:, gcol:gcol + 1], axis=0))
                            for k in range(2):
                                tp = ps.tile([128, 128], f32, tag="ps_tr")
                                nc.tensor.transpose(out=tp[:], in_=g[:, k * 128:(k + 1) * 128],
                                                    identity=ident[:])
                                nc.vector.tensor_copy(out=xT[:, koff + k, s * 128:(s + 1) * 128], in_=tp[:])
                        # logits -> exp
                        plg = ps.tile([128, 2], f32, tag="ps_lg", bufs=1)
                        for k in range(2):
                            nc.tensor.matmul(plg[:], xT[:, k, s * 128:(s + 1) * 128],
                                             wcat_sb[:, k, :], start=(k == 0), stop=(k == 1))
                        exps = sb.tile([128, 2], f32, tag="exps", bufs=6)
                        nc.scalar.activation(out=exps[:, 0:1], in_=plg[:, 0:1], func=A.Exp,
                                             bias=ab_bc[:, 0:1], scale=1.0)
                        nc.scalar.activation(out=exps[:, 1:2], in_=plg[:, 1:2], func=A.Exp,
                                             bias=ab_bc[:, 1:2], scale=1.0)
                        sub_data.append((e0, exps))
                    h3 = _mlp(nc, (sb, ps), [xT[:, k, :] for k in range(6)], eW, eB, 6, 512)
                    for s in range(4):
                        e0, exps = sub_data[s]
                        ups = ps.tile([128, D], f32, tag="ps_ups", bufs=1)
                        for k in range(2):
                            tp2 = ps.tile([128, 128], f32, tag="ps_tr")
                            nc.tensor.transpose(out=tp2[:], in_=h3[:, k, s * 128:(s + 1) * 128],
                                                identity=ident[:])
                            nc.vector.tensor_copy(out=ups[:, k * 128:(k + 1) * 128], in_=tp2[:])
                        u = _ln_from_psum(nc, sb, ups, eg_bc, ebt_bc, eps_ln)
                        # residual edge output
                        er = sb.tile([128, D], f32, tag="er", bufs=4)
                        nc.sync.dma_start(out=er[:], in_=edges_r[e0:e0 + 128, :])
                        oe = sb.tile([128, D], f32, tag="oe", bufs=4)
                        nc.vector.tensor_tensor(out=oe[:], in0=u[:], in1=er[:], op=mybir.AluOpType.add)
                        nc.scalar.dma_start(out=out_edges[e0:e0 + 128, :], in_=oe[:])
                        # pack [U | 1 | exp_s] bf16 and scatter to pass-2 scratch
                        ub = sb.tile([128, 258], bf16, tag="ub", bufs=4)
                        nc.vector.tensor_copy(out=ub[:, 0:256], in_=u[:])
                        nc.vector.memset(ub[:, 256:257], 1.0)
                        nc.vector.tensor_copy(out=ub[:, 257:258], in_=exps[:, 1:2])
                        sd = sb.tile([128, 1], i32, tag="sd", bufs=6)
                        nc.sync.dma_start(out=sd[:, 0:1], in_=sdest_in[e0:e0 + 128, None])
                        nc.gpsimd.indirect_dma_start(
                            out=scratch[:], out_offset=bass.IndirectOffsetOnAxis(ap=sd[:, :1], axis=0),
                            in_=ub[:], in_offset=None,
                            bounds_check=E_PAD2 - 1, oob_is_err=False)
                        # receiver one-hot aggregation
                        rs = sb.tile([128, 1], f32, tag="rs", bufs=6)
                        nc.sync.dma_start(out=rs[:, 0:1], in_=rslot_in[e0:e0 + 128, None])
                        oh = sb.tile([128, 128], bf16, tag="oh", bufs=4)
                        nc.vector.tensor_tensor(out=oh[:], in0=rs[:, :1].to_broadcast([128, 128]),
                                                in1=iota_f[:], op=mybir.AluOpType.is_equal)
                        S = sb.tile([128, 128], bf16, tag="S", bufs=4)
                        nc.vector.tensor_scalar_mul(out=S[:], in0=oh[:], scalar1=exps[:, 0:1])
                        first = (sup == 0 and s == 0)
                        last = (sup == n_super - 1 and s == 3)
                        nc.tensor.matmul(ps_win[:], S[:], ub[:, 0:257], start=first, stop=last)
                # window close: normalize and transpose into recvT
                den = sb.tile([128, 1], f32, tag="den")
                nc.vector.tensor_tensor(out=den[:], in0=ps_win[:, 256:257], in1=eps30[:],
                                        op=mybir.AluOpType.add)
                nc.vector.reciprocal(out=den[:], in_=den[:])
                agg = sb.tile([128, D], f32, tag="agg")
                nc.vector.tensor_scalar_mul(out=agg[:], in0=ps_win[:, 0:256], scalar1=den[:, :1])
                nc.scalar.dma_start(out=dbg_ragg[lw * 128:(lw + 1) * 128, :], in_=agg[:])
                for k in range(2):
                    tp3 = ps.tile([128, 128], f32, tag="ps_tr")
                    nc.tensor.transpose(out=tp3[:], in_=agg[:, k * 128:(k + 1) * 128], identity=ident[:])
                    nc.vector.tensor_copy(out=recvT[:, k, lw * 128:(lw + 1) * 128], in_=tp3[:])

        # ================= PASS 2: sender aggregation =================
        with tc.tile_pool(name="p2sb", bufs=3) as sb, \
             tc.tile_pool(name="p2ps", bufs=2, space="PSUM") as ps:
            sc_v = scratch[:].rearrange("(t p) c -> p t c", p=128)
            for w in range(N_WIN):
                pw = ps.tile([128, 257], f32, tag="p2win")
                for i in range(TPW2):
                    t2 = w * TPW2 + i
                    sct = sb.tile([128, 258], bf16, tag="sct")
                    nc.sync.dma_start(out=sct[:], in_=sc_v[:, t2, :])
                    sl = sb.tile([128, 1], f32, tag="sl")
                    nc.sync.dma_start(out=sl[:, 0:1], in_=p2slot_in[t2 * 128:(t2 + 1) * 128, None])
                    oh2 = sb.tile([128, 128], bf16, tag="oh2")
                    nc.vector.tensor_tensor(out=oh2[:], in0=sl[:, :1].to_broadcast([128, 128]),
                                            in1=iota_f[:], op=mybir.AluOpType.is_equal)
                    exf = sb.tile([128, 1], f32, tag="exf")
                    nc.vector.tensor_copy(out=exf[:], in_=sct[:, 257:258])
                    S2 = sb.tile([128, 128], bf16, tag="S2")
                    nc.vector.tensor_scalar_mul(out=S2[:], in0=oh2[:], scalar1=exf[:, :1])
                    nc.tensor.matmul(pw[:], S2[:], sct[:, 0:257], start=(i == 0), stop=(i == TPW2 - 1))
                po = sb.tile([128, 257], f32, tag="po")
                nc.vector.tensor_copy(out=po[:], in_=pw[:])
                nc.sync.dma_start(out=send_part[w * 128:(w + 1) * 128, :], in_=po[:])

        # ================= ReduceScatter + node MLP =================
        nc.gpsimd.collective_compute(
            "ReduceScatter", mybir.AluOpType.add,
            replica_groups=[list(range(N_CORES))],
            ins=[send_part.opt()], outs=[rs_out.opt()])

        with tc.tile_pool(name="p3sb", bufs=2) as sb, \
             tc.tile_pool(name="p3ps", bufs=2, space="PSUM") as ps:
            for nw in range(WIN_PER_CORE):
                rst = sb.tile([128, 257], f32, tag="rst")
                nc.sync.dma_start(out=rst[:], in_=rs_out[nw * 128:(nw + 1) * 128, :])
                den = sb.tile([128, 1], f32, tag="den3")
                nc.vector.tensor_tensor(out=den[:], in0=rst[:, 256:257], in1=eps30[:],
                                        op=mybir.AluOpType.add)
                nc.vector.reciprocal(out=den[:], in_=den[:])
                sagg = sb.tile([128, D], f32, tag="sagg")
                nc.vector.tensor_scalar_mul(out=sagg[:], in0=rst[:, 0:256], scalar1=den[:, :1])
                nc.scalar.dma_start(out=dbg_sagg[nw * 128:(nw + 1) * 128, :], in_=sagg[:])
                for k in range(2):
                    tp = ps.tile([128, 128], f32, tag="ps_tr3")
                    nc.tensor.transpose(out=tp[:], in_=sagg[:, k * 128:(k + 1) * 128], identity=ident[:])
                    nc.vector.tensor_copy(out=sentT[:, k, nw * 128:(nw + 1) * 128], in_=tp[:])
            nodes_t_v = nodes_tc.rearrange("(k p) e -> p k e", p=128)
            for c0 in range(0, CHUNK, 512):
                wid = min(512, CHUNK - c0)
                nT = sb.tile([128, 2, wid], f32r, tag="nT")
                nc.sync.dma_start(out=nT[:], in_=nodes_t_v[:, :, c0:c0 + wid])
                srcs = ([nT[:, k, :] for k in range(2)]
                        + [recvT[:, k, c0:c0 + wid] for k in range(2)]
                        + [sentT[:, k, c0:c0 + wid] for k in range(2)])
                h3 = _mlp(nc, (sb, ps), srcs, nW, nB, 6, wid)
                for s in range(wid // 128):
                    n0 = c0 + s * 128
                    ups = ps.tile([128, D], f32, tag="ps_ups3")
                    for k in range(2):
                        tp2 = ps.tile([128, 128], f32, tag="ps_tr3")
                        nc.tensor.transpose(out=tp2[:], in_=h3[:, k, s * 128:(s + 1) * 128],
                                            identity=ident[:])
                        nc.vector.tensor_copy(out=ups[:, k * 128:(k + 1) * 128], in_=tp2[:])
                    un = _ln_from_psum(nc, sb, ups, ng_bc, nbt_bc, eps_ln)
                    nr = sb.tile([128, D], f32, tag="nr")
                    nc.sync.dma_start(out=nr[:], in_=nodes_rc[n0:n0 + 128, :])
                    on = sb.tile([128, D], f32, tag="on")
                    nc.vector.tensor_tensor(out=on[:], in0=un[:], in1=nr[:], op=mybir.AluOpType.add)
                    nc.scalar.dma_start(out=out_nodes[n0:n0 + 128, :], in_=on[:])

        consts.release()
        dram.release()

    nc.compile()
    return nc


# ----------------------------------------------------------------------------
# entry point
# ----------------------------------------------------------------------------

def kernel(nodes, edges, senders, receivers,
           edge_W0, edge_b0, edge_W1, edge_b1, edge_W2, edge_b2, edge_ln_g, edge_ln_b,
           node_W0, node_b0, node_W1, node_b1, node_W2, node_b2, node_ln_g, node_ln_b,
           recv_attn_w, recv_attn_b, send_attn_w, send_attn_b):
    global LAST_EXEC_NS
    import os
    nodes = np.asarray(nodes, dtype=np.float32)
    edges = np.asarray(edges, dtype=np.float32)
    slot_of_node, nodes_rl, cores, TPW, E_PAD, TPW2, E_PAD2 = _preprocess(
        nodes, edges, senders, receivers)
    nc = build_program(TPW, E_PAD, TPW2, E_PAD2)

    wcat = np.concatenate([np.asarray(recv_attn_w), np.asarray(send_attn_w)], axis=1).astype(np.float32)
    attn_b = np.concatenate([np.asarray(recv_attn_b), np.asarray(send_attn_b)]).astype(np.float32)
    shared = dict(
        nodes_rl=nodes_rl, wcat=wcat, attn_b=attn_b,
        eW0=np.asarray(edge_W0, np.float32), eW1=np.asarray(edge_W1, np.float32),
        eW2=np.asarray(edge_W2, np.float32),
        eb0=np.asarray(edge_b0, np.float32), eb1=np.asarray(edge_b1, np.float32),
        eb2=np.asarray(edge_b2, np.float32),
        eg=np.asarray(edge_ln_g, np.float32), ebt=np.asarray(edge_ln_b, np.float32),
        nW0=np.asarray(node_W0, np.float32), nW1=np.asarray(node_W1, np.float32),
        nW2=np.asarray(node_W2, np.float32),
        nb0=np.asarray(node_b0, np.float32), nb1=np.asarray(node_b1, np.float32),
        nb2=np.asarray(node_b2, np.float32),
        ng=np.asarray(node_ln_g, np.float32), nbt=np.asarray(node_ln_b, np.float32),
    )
    in_maps = []
    for c in range(N_CORES):
        m = dict(shared)
        for k in ("edges_t", "edges_r", "meta", "rslot", "sdest", "p2slot",
                  "nodes_r_chunk", "nodes_t_chunk"):
            tgt = {"nodes_r_chunk": "nodes_rc", "nodes_t_chunk": "nodes_tc"}.get(k, k)
            m[tgt] = cores[c][k]
        in_maps.append(m)

    trace = bool(int(os.environ.get("KERNEL_TRACE", "0")))
    res = run_bass_kernel_spmd(nc, in_maps, list(range(N_CORES)), trace=trace)
    LAST_EXEC_NS = res.exec_time_ns
    global LAST_RESULTS, LAST_PP
    LAST_RESULTS = res.results
    LAST_PP = (slot_of_node, cores)

    edges_out = np.empty((N_EDGES, D), dtype=np.float32)
    nodes_out = np.empty((N_NODES, D), dtype=np.float32)
    inv_slot = np.full(N_SLOTS, -1, dtype=np.int64)
    inv_slot[slot_of_node] = np.arange(N_NODES)
    for c in range(N_CORES):
        eo = res.results[c]["out_edges"]
        ids = cores[c]["edge_ids"]
        real = ids >= 0
        edges_out[ids[real]] = eo[real]
        no = res.results[c]["out_nodes"]
        sl = inv_slot[c * CHUNK:(c + 1) * CHUNK]
        rm = sl >= 0
        nodes_out[sl[rm]] = no[rm]
    return nodes_out, edges_out


# revision 22
# speedup vs baseline: 1.2016x; 1.1498x over previous
"""AttentionInteractionNetwork GNN on 8 Trainium2 NeuronCores.

Sharding: edges partitioned by (relabeled) receiver so each core owns a
contiguous 1280-slot node chunk; receiver aggregation is core-local, the
sender aggregation partials are ReduceScattered. MLP matmuls run in
float32r (full PE rate); one-hot scatter matmuls run in bf16.
"""

import sys

sys.path.insert(0, "/opt/trn_rl_repo")

import numpy as np

import concourse.bass as bass
import concourse.bacc as bacc
import concourse.mybir as mybir
import concourse.tile as tile
from concourse.masks import make_identity
from concourse.bass_utils import run_bass_kernel_spmd

N_NODES = 10000
N_EDGES = 320000
D = 256
H = 512
EPS = 1e-5
N_CORES = 8
N_WIN = 80                 # 128-slot node windows
N_SLOTS = N_WIN * 128      # 10240 relabeled node slots
WIN_PER_CORE = N_WIN // N_CORES
CHUNK = WIN_PER_CORE * 128  # 1280 slots per core
DUMMY_SLOT = N_SLOTS - 1    # guaranteed dummy (10240 > 10000 real nodes)

f32 = mybir.dt.float32
f32r = mybir.dt.float32r
bf16 = mybir.dt.bfloat16
i32 = mybir.dt.int32

LAST_EXEC_NS = None
LAST_RESULTS = None
LAST_PP = None


# ----------------------------------------------------------------------------
# host-side preprocessing
# ----------------------------------------------------------------------------

def _relabel_nodes(receivers):
    """LPT-assign nodes to N_WIN windows of <=125 real nodes balancing
    in-degree. Returns slot_of_node [N_NODES] (node -> slot id)."""
    indeg = np.bincount(receivers, minlength=N_NODES)
    order = np.argsort(-indeg, kind="stable")
    win_load = np.zeros(N_WIN, dtype=np.int64)
    win_cnt = np.zeros(N_WIN, dtype=np.int64)
    win_members = [[] for _ in range(N_WIN)]
    # max real nodes per window so every window keeps some dummies is not
    # required; cap at 128.
    import heapq

    heap = [(0, 0, w) for w in range(N_WIN)]
    heapq.heapify(heap)
    for n in order:
        while True:
            load, cnt, w = heapq.heappop(heap)
            if win_cnt[w] < 128:
                break
        win_members[w].append(n)
        win_load[w] += indeg[n]
        win_cnt[w] += 1
        heapq.heappush(heap, (int(win_load[w]), int(win_cnt[w]), w))
    slot_of_node = np.full(N_NODES, -1, dtype=np.int64)
    for w in range(N_WIN):
        for j, n in enumerate(win_members[w]):
            slot_of_node[n] = w * 128 + j
    assert (slot_of_node >= 0).all()
    return slot_of_node


def _ceil_to(x, m):
    return ((x + m - 1) // m) * m


def _preprocess(nodes, edges, senders, receivers):
    senders = np.asarray(senders).astype(np.int64)
    receivers = np.asarray(receivers).astype(np.int64)
    slot_of_node = _relabel_nodes(receivers)
    r_slot = slot_of_node[receivers]   # [E]
    s_slot = slot_of_node[senders]     # [E]
    r_win = r_slot >> 7                # receiver window = core*10 + local
    core_of_edge = r_win // WIN_PER_CORE

    # --- pass-1 layout: per core, edges sorted by receiver window, each
    # window padded to TPW tiles of 128 ---
    per_cw_idx = [[None] * N_WIN for _ in range(N_CORES)]
    max_cnt = 0
    for c in range(N_CORES):
        em = np.nonzero(core_of_edge == c)[0]
        w_of = r_win[em]
        order = np.argsort(w_of, kind="stable")
        em = em[order]
        w_of = w_of[order]
        bounds = np.searchsorted(w_of, np.arange(c * WIN_PER_CORE, (c + 1) * WIN_PER_CORE + 1))
        for lw in range(WIN_PER_CORE):
            idx = em[bounds[lw]:bounds[lw + 1]]
            per_cw_idx[c][c * WIN_PER_CORE + lw] = idx
            max_cnt = max(max_cnt, len(idx))
    TPW = _ceil_to(max(_ceil_to(max_cnt, 128) // 128, 4), 4)  # tiles/window, mult of 4
    E_PAD = WIN_PER_CORE * TPW * 128

    # --- pass-2 layout: per core, edges grouped by sender window, each
    # group padded to TPW2 tiles of 128 ---
    max_cnt2 = 0
    per_c_sgroups = []
    for c in range(N_CORES):
        em = np.concatenate([per_cw_idx[c][c * WIN_PER_CORE + lw] for lw in range(WIN_PER_CORE)])
        sw = s_slot[em] >> 7
        order = np.argsort(sw, kind="stable")
        em2 = em[order]
        sw2 = sw[order]
        bounds = np.searchsorted(sw2, np.arange(N_WIN + 1))
        per_c_sgroups.append((em2, bounds))
        max_cnt2 = max(max_cnt2, int(np.max(bounds[1:] - bounds[:-1])))
    TPW2 = max(_ceil_to(max_cnt2, 128) // 128, 1)
    E_PAD2 = N_WIN * TPW2 * 128

    # --- per-core arrays ---
    pos_in_p2 = np.empty(N_EDGES, dtype=np.int64)  # global edge -> pass2 pos (per its core)
    cores = []
    nodes_rl = np.zeros((N_SLOTS, D), dtype=np.float32)
    nodes_rl[slot_of_node] = np.asarray(nodes, dtype=np.float32)
    for c in range(N_CORES):
        em2, bounds = per_c_sgroups[c]
        for w in range(N_WIN):
            seg = em2[bounds[w]:bounds[w + 1]]
            pos_in_p2[seg] = w * TPW2 * 128 + np.arange(len(seg))

        edge_ids = np.full(E_PAD, -1, dtype=np.int64)
        for lw in range(WIN_PER_CORE):
            idx = per_cw_idx[c][c * WIN_PER_CORE + lw]
            base = lw * TPW * 128
            edge_ids[base:base + len(idx)] = idx
        real = edge_ids >= 0
        eidx = edge_ids[real]

        e_feat = np.zeros((E_PAD, D), dtype=np.float32)
        e_feat[real] = np.asarray(edges, dtype=np.float32)[eidx]
        meta = np.empty((E_PAD, 4), dtype=np.int32)
        meta[:, 0] = DUMMY_SLOT; meta[:, 1] = DUMMY_SLOT
        meta[:, 2] = 1 << 30; meta[:, 3] = -1
        meta[real, 0] = s_slot[eidx].astype(np.int32)
        meta[real, 1] = r_slot[eidx].astype(np.int32)
        meta[real, 2] = pos_in_p2[eidx].astype(np.int32)
        meta[real, 3] = (r_slot[eidx] & 127).astype(np.int32)
        rslot = np.full(E_PAD, -1.0, dtype=np.float32)
        rslot[real] = (r_slot[eidx] & 127).astype(np.float32)
        sdest = np.full(E_PAD, 1 << 30, dtype=np.int32)
        sdest[real] = pos_in_p2[eidx].astype(np.int32)
        p2slot = np.full(E_PAD2, -1.0, dtype=np.float32)
        p2slot[pos_in_p2[eidx]] = (s_slot[eidx] & 127).astype(np.float32)

        cores.append(dict(
            edge_ids=edge_ids,
            edges_t=np.ascontiguousarray(e_feat.T),
            edges_r=e_feat,
            meta=meta, rslot=rslot, sdest=sdest, p2slot=p2slot,
            nodes_r_chunk=nodes_rl[c * CHUNK:(c + 1) * CHUNK],
            nodes_t_chunk=np.ascontiguousarray(nodes_rl[c * CHUNK:(c + 1) * CHUNK].T),
        ))
    return slot_of_node, nodes_rl, cores, TPW, E_PAD, TPW2, E_PAD2


# ----------------------------------------------------------------------------
# device program
# ----------------------------------------------------------------------------

def _mlp(nc, pools, xT_srcs, W_sb, b_sb, ktiles, width):
    """Feature-major 768->512->512->256 MLP. xT_srcs: list of 6 (tile, kslice)
    rhs sources [128, width]. Returns h3T sbuf tile [128, 2, width] (f32)."""
    sb, ps = pools
    W0, W1, W2 = W_sb
    b0, b1, b2 = b_sb
    h1 = sb.tile([128, 4, width], f32r, tag="h1")
    for m in range(4):
        p1 = ps.tile([128, width], f32, tag="ps_mlp", bufs=3)
        for k in range(6):
            src = xT_srcs[k]
            nc.tensor.matmul(p1[:], W0[:, k, m * 128:(m + 1) * 128], src,
                             start=(k == 0), stop=(k == 5))
        nc.scalar.activation(out=h1[:, m, :], in_=p1[:],
                             func=mybir.ActivationFunctionType.Silu,
                             bias=b0[:, m:m + 1], scale=1.0)
    h2 = sb.tile([128, 4, width], f32r, tag="h2")
    for m in range(4):
        p2 = ps.tile([128, width], f32, tag="ps_mlp", bufs=3)
        for k in range(4):
            nc.tensor.matmul(p2[:], W1[:, k, m * 128:(m + 1) * 128], h1[:, k, :],
                             start=(k == 0), stop=(k == 3))
        nc.scalar.activation(out=h2[:, m, :], in_=p2[:],
                             func=mybir.ActivationFunctionType.Silu,
                             bias=b1[:, m:m + 1], scale=1.0)
    h3 = sb.tile([128, 2, width], f32, tag="h3")
    for m in range(2):
        p3 = ps.tile([128, width], f32, tag="ps_mlp", bufs=3)
        for k in range(4):
            nc.tensor.matmul(p3[:], W2[:, k, m * 128:(m + 1) * 128], h2[:, k, :],
                             start=(k == 0), stop=(k == 3))
        nc.vector.tensor_scalar_add(out=h3[:, m, :], in0=p3[:], scalar1=b2[:, m:m + 1])
    return h3


def _ln_from_psum(nc, sb, u_ps, g_bc, b_bc, eps_t, width=D):
    """LayerNorm over free axis of u_ps [128, width] -> sbuf f32 tile."""
    stats = sb.tile([128, 6], f32, tag="ln_stats")
    nc.vector.bn_stats(out=stats[:], in_=u_ps[:])
    mv = sb.tile([128, 2], f32, tag="ln_mv")
    nc.vector.bn_aggr(out=mv[:], in_=stats[:])
    rstd = sb.tile([128, 1], f32, tag="ln_rstd")
    nc.scalar.activation(out=rstd[:], in_=mv[:, 1:2],
                         func=mybir.ActivationFunctionType.Sqrt,
                         bias=eps_t[:], scale=1.0)
    nc.vector.reciprocal(out=rstd[:], in_=rstd[:])
    u = sb.tile([128, width], f32, tag="ln_out")
    nc.vector.tensor_scalar(out=u[:], in0=u_ps[:], scalar1=mv[:, 0:1],
                            scalar2=rstd[:, :1],
                            op0=mybir.AluOpType.subtract, op1=mybir.AluOpType.mult)
    nc.vector.tensor_tensor(out=u[:], in0=u[:], in1=g_bc[:], op=mybir.AluOpType.mult)
    nc.vector.tensor_tensor(out=u[:], in0=u[:], in1=b_bc[:], op=mybir.AluOpType.add)
    return u


def build_program(TPW, E_PAD, TPW2, E_PAD2):
    nc = bacc.Bacc("TRN2", target_bir_lowering=False, debug=False, num_devices=N_CORES)
    A = mybir.ActivationFunctionType

    def din(name, shape, dt):
        return nc.dram_tensor(name, shape, dt, kind="ExternalInput").ap()

    edges_t = din("edges_t", [D, E_PAD], f32r)
    edges_r = din("edges_r", [E_PAD, D], f32)
    nodes_rl = din("nodes_rl", [N_SLOTS, D], f32)
    nodes_rc = din("nodes_rc", [CHUNK, D], f32)
    nodes_tc = din("nodes_tc", [D, CHUNK], f32r)
    meta_in = din("meta", [E_PAD, 4], i32)
    rslot_in = din("rslot", [E_PAD], f32)
    sdest_in = din("sdest", [E_PAD], i32)
    p2slot_in = din("p2slot", [E_PAD2], f32)
    wcat = din("wcat", [D, 2], f32r)
    attn_b = din("attn_b", [2], f32)
    eW0 = din("eW0", [3 * D, H], f32r); eW1 = din("eW1", [H, H], f32r); eW2 = din("eW2", [H, D], f32r)
    eb0 = din("eb0", [H], f32); eb1 = din("eb1", [H], f32); eb2 = din("eb2", [D], f32)
    eg = din("eg", [D], f32); ebt = din("ebt", [D], f32)
    nW0 = din("nW0", [3 * D, H], f32r); nW1 = din("nW1", [H, H], f32r); nW2 = din("nW2", [H, D], f32r)
    nb0 = din("nb0", [H], f32); nb1 = din("nb1", [H], f32); nb2 = din("nb2", [D], f32)
    ng = din("ng", [D], f32); nbt = din("nbt", [D], f32)

    out_edges = nc.dram_tensor("out_edges", [E_PAD, D], f32, kind="ExternalOutput").ap()
    out_nodes = nc.dram_tensor("out_nodes", [CHUNK, D], f32, kind="ExternalOutput").ap()
    dbg_ragg = nc.dram_tensor("dbg_ragg", [CHUNK, D], f32, kind="ExternalOutput").ap()
    dbg_sagg = nc.dram_tensor("dbg_sagg", [CHUNK, D], f32, kind="ExternalOutput").ap()

    T2 = N_WIN * TPW2

    with tile.TileContext(nc) as tc:
        consts = tc.alloc_tile_pool(name="consts", bufs=1)
        dram = tc.alloc_tile_pool(name="dram", bufs=1, space="DRAM")

        _uid = [0]

        def _tag(p):
            _uid[0] += 1
            return f"{p}{_uid[0]}"

        def load_w(ap, kt, mwid):
            t = consts.tile([128, kt, mwid], f32r, tag=_tag("w"))
            nc.sync.dma_start(out=t[:], in_=ap.rearrange("(k p) m -> p k m", p=128))
            return t

        def load_b(ap, mt):
            t = consts.tile([128, mt], f32, tag=_tag("b"))
            nc.sync.dma_start(out=t[:], in_=ap.rearrange("(m p) -> p m", p=128))
            return t

        def load_bc(ap, width):
            t = consts.tile([128, width], f32, tag=_tag("bc"))
            src = bass.AP(tensor=ap.tensor, offset=ap.offset, ap=[[0, 128]] + list(ap.ap))
            nc.gpsimd.dma_start(out=t[:], in_=src)
            return t

        eW = (load_w(eW0, 6, H), load_w(eW1, 4, H), load_w(eW2, 4, D))
        nW = (load_w(nW0, 6, H), load_w(nW1, 4, H), load_w(nW2, 4, D))
        eB = (load_b(eb0, 4), load_b(eb1, 4), load_b(eb2, 2))
        nB = (load_b(nb0, 4), load_b(nb1, 4), load_b(nb2, 2))
        eg_bc = load_bc(eg, D); ebt_bc = load_bc(ebt, D)
        ng_bc = load_bc(ng, D); nbt_bc = load_bc(nbt, D)
        wcat_sb = consts.tile([128, 2, 2], f32r)
        nc.sync.dma_start(out=wcat_sb[:], in_=wcat.rearrange("(k p) t -> p k t", p=128))
        ab_bc = load_bc(attn_b, 2)  # [128, 2]: col0 = recv bias, col1 = send bias
        iota_i = consts.tile([128, 128], i32)
        nc.gpsimd.iota(iota_i[:], pattern=[[1, 128]], base=0, channel_multiplier=0)
        iota_f = consts.tile([128, 128], f32)
        nc.vector.tensor_copy(out=iota_f[:], in_=iota_i[:])
        ident = consts.tile([128, 128], f32)
        make_identity(nc, ident[:])
        recvT = consts.tile([128, 2, CHUNK], f32r)   # recv_agg feature-major
        sentT = consts.tile([128, 2, CHUNK], f32r)   # sent_agg feature-major
        eps30 = consts.tile([128, 1], f32)
        nc.vector.memset(eps30[:], 1e-30)
        eps_ln = consts.tile([128, 1], f32)
        nc.vector.memset(eps_ln[:], EPS)

        scratch = dram.tile([E_PAD2, 258], bf16)
        send_part = dram.tile([N_SLOTS, 257], f32)
        rs_out = dram.tile([CHUNK, 257], f32)

        # ---- memset pass-2 scratch ----
        with tc.tile_pool(name="ms", bufs=2) as ms:
            z = ms.tile([128, 8, 258], bf16)
            nc.vector.memset(z[:], 0.0)
            sview = scratch[:].rearrange("(t p) c -> p t c", p=128)
            for i in range(0, T2, 8):
                nc.sync.dma_start(out=sview[:, i:i + 8, :], in_=z[:])

        # ================= PASS 1: edges =================
        with tc.tile_pool(name="p1sb", bufs=2) as sb, \
             tc.tile_pool(name="p1ps", bufs=2, space="PSUM") as ps, \
             tc.tile_pool(name="p1win", bufs=1, space="PSUM") as psw:
            edges_t_v = edges_t.rearrange("(k p) e -> p k e", p=128)
            n_super = TPW // 4
            for lw in range(WIN_PER_CORE):
                ps_win = psw.tile([128, 257], f32, tag="win")
                for sup in range(n_super):
                    st0 = (lw * TPW + sup * 4) * 128  # first edge of super-tile
                    xT = sb.tile([128, 6, 512], f32r, tag="xT", bufs=3)
                    nc.sync.dma_start(out=xT[:, 0:2, :], in_=edges_t_v[:, :, st0:st0 + 512])
                    sub_data = []
                    for s in range(4):
                        e0 = st0 + s * 128
                        gi = sb.tile([128, 4], i32, tag="gi", bufs=12)
                        nc.sync.dma_start(out=gi[:], in_=meta_in[e0:e0 + 128, :])
                        for gcol, koff in ((0, 2), (1, 4)):
                            g = sb.tile([128, D], f32, tag="gath", bufs=10)
                            nc.gpsimd.indirect_dma_start(
                                out=g[:], out_offset=None, in_=nodes_rl[:],
                                in_offset=bass.IndirectOffsetOnAxis(ap=gi[:, gcol:gcol + 1], axis=0))
                            for k in range(2):
                                tp = ps.tile([128, 128], f32, tag="ps_tr")
                                nc.tensor.transpose(out=tp[:], in_=g[:, k * 128:(k + 1) * 128],
                                                    identity=ident[:])
                                nc.vector.tensor_copy(out=xT[:, koff + k, s * 128:(s + 1) * 128], in_=tp[:])
                        # logits -> exp
                        plg = ps.tile([128, 2], f32, tag="ps_lg", bufs=1)
                        for k in range(2):
                            nc.tensor.matmul(plg[:], xT[:, k, s * 128:(s + 1) * 128],
                                             wcat_sb[:, k, :], start=(k == 0), stop=(k == 1))
                        exps = sb.tile([128, 2], f32, tag="exps", bufs=6)
                        nc.scalar.activation(out=exps[:, 0:1], in_=plg[:, 0:1], func=A.Exp,
                                             bias=ab_bc[:, 0:1], scale=1.0)
                        nc.scalar.activation(out=exps[:, 1:2], in_=plg[:, 1:2], func=A.Exp,
                                             bias=ab_bc[:, 1:2], scale=1.0)
                        sub_data.append((e0, exps))
                    h3 = _mlp(nc, (sb, ps), [xT[:, k, :] for k in range(6)], eW, eB, 6, 512)
                    for s in range(4):
                        e0, exps = sub_data[s]
                        ups = ps.tile([128, D], f32, tag="ps_ups", bufs=1)
                        for k in range(2):
                            tp2 = ps.tile([128, 128], f32, tag="ps_tr")
                            nc.tensor.transpose(out=tp2[:], in_=h3[:, k, s * 128:(s + 1) * 128],
                                                identity=ident[:])
                            nc.vector.tensor_copy(out=ups[:, k * 128:(k + 1) * 128], in_=tp2[:])
                        u = _ln_from_psum(nc, sb, ups, eg_bc, ebt_bc, eps_ln)
                        # residual edge output
                        er = sb.tile([128, D], f32, tag="er", bufs=4)
                        nc.sync.dma_start(out=er[:], in_=edges_r[e0:e0 + 128, :])
                        oe = sb.tile([128, D], f32, tag="oe", bufs=4)
                        nc.vector.tensor_tensor(out=oe[:], in0=u[:], in1=er[:], op=mybir.AluOpType.add)
                        nc.scalar.dma_start(out=out_edges[e0:e0 + 128, :], in_=oe[:])
                        # pack [U | 1 | exp_s] bf16 and scatter to pass-2 scratch
                        ub = sb.tile([128, 258], bf16, tag="ub", bufs=4)
                        nc.vector.tensor_copy(out=ub[:, 0:256], in_=u[:])
                        nc.vector.memset(ub[:, 256:257], 1.0)
                        nc.vector.tensor_copy(out=ub[:, 257:258], in_=exps[:, 1:2])
                        sd = sb.tile([128, 1], i32, tag="sd", bufs=6)
                        nc.sync.dma_start(out=sd[:, 0:1], in_=sdest_in[e0:e0 + 128, None])
                        nc.gpsimd.indirect_dma_start(
                            out=scratch[:], out_offset=bass.IndirectOffsetOnAxis(ap=sd[:, :1], axis=0),
                            in_=ub[:], in_offset=None,
                            bounds_check=E_PAD2 - 1, oob_is_err=False)
                        # receiver one-hot aggregation
                        rs = sb.tile([128, 1], f32, tag="rs", bufs=6)
                        nc.sync.dma_start(out=rs[:, 0:1], in_=rslot_in[e0:e0 + 128, None])
                        oh = sb.tile([128, 128], bf16, tag="oh", bufs=4)
                        nc.vector.tensor_tensor(out=oh[:], in0=rs[:, :1].to_broadcast([128, 128]),
                                                in1=iota_f[:], op=mybir.AluOpType.is_equal)
                        S = sb.tile([128, 128], bf16, tag="S", bufs=4)
                        nc.vector.tensor_scalar_mul(out=S[:], in0=oh[:], scalar1=exps[:, 0:1])
                        first = (sup == 0 and s == 0)
                        last = (sup == n_super - 1 and s == 3)
                        nc.tensor.matmul(ps_win[:], S[:], ub[:, 0:257], start=first, stop=last)
                # window close: normalize and transpose into recvT
                den = sb.tile([128, 1], f32, tag="den")
                nc.vector.tensor_tensor(out=den[:], in0=ps_win[:, 256:257], in1=eps30[:],
                                        op=mybir.AluOpType.add)
                nc.vector.reciprocal(out=den[:], in_=den[:])
                agg = sb.tile([128, D], f32, tag="agg")
                nc.vector.tensor_scalar_mul(out=agg[:], in0=ps_win[:, 0:256], scalar1=den[:, :1])
                nc.scalar.dma_start(out=dbg_ragg[lw * 128:(lw + 1) * 128, :], in_=agg[:])
                for k in range(2):
                    tp3 = ps.tile([128, 128], f32, tag="ps_tr")
                    nc.tensor.transpose(out=tp3[:], in_=agg[:, k * 128:(k + 1) * 128], identity=ident[:])
                    nc.vector.tensor_copy(out=recvT[:, k, lw * 128:(lw + 1) * 128], in_=tp3[:])

        # ================= PASS 2: sender aggregation =================
        with tc.tile_pool(name="p2sb", bufs=3) as sb, \
             tc.tile_pool(name="p2ps", bufs=2, space="PSUM") as ps:
            sc_v = scratch[:].rearrange("(t p) c -> p t c", p=128)
            p2s_v = p2slot_in.rearrange("(w t p) -> p w t", p=128, w=N_WIN)
            for w in range(N_WIN):
                pw = ps.tile([128, 257], f32, tag="p2win")
                sctw = sb.tile([128, TPW2, 258], bf16, tag="sctw", bufs=3)
                nc.sync.dma_start(out=sctw[:], in_=sc_v[:, w * TPW2:(w + 1) * TPW2, :])
                slw = sb.tile([128, TPW2], f32, tag="slw", bufs=3)
                nc.sync.dma_start(out=slw[:], in_=p2s_v[:, w, :])
                for i in range(TPW2):
                    oh2 = sb.tile([128, 128], bf16, tag="oh2")
                    nc.vector.tensor_tensor(out=oh2[:], in0=slw[:, i:i + 1].to_broadcast([128, 128]),
                                            in1=iota_f[:], op=mybir.AluOpType.is_equal)
                    exf = sb.tile([128, 1], f32, tag="exf")
                    nc.vector.tensor_copy(out=exf[:], in_=sctw[:, i, 257:258])
                    S2 = sb.tile([128, 128], bf16, tag="S2")
                    nc.vector.tensor_scalar_mul(out=S2[:], in0=oh2[:], scalar1=exf[:, :1])
                    nc.tensor.matmul(pw[:], S2[:], sctw[:, i, 0:257], start=(i == 0), stop=(i == TPW2 - 1))
                po = sb.tile([128, 257], f32, tag="po")
                nc.vector.tensor_copy(out=po[:], in_=pw[:])
                nc.sync.dma_start(out=send_part[w * 128:(w + 1) * 128, :], in_=po[:])

        # ================= ReduceScatter + node MLP =================
        nc.gpsimd.collective_compute(
            "ReduceScatter", mybir.AluOpType.add,
            replica_groups=[list(range(N_CORES))],
            ins=[send_part.opt()], outs=[rs_out.opt()])

        with tc.tile_pool(name="p3sb", bufs=2) as sb, \
             tc.tile_pool(name="p3ps", bufs=2, space="PSUM") as ps:
            for nw in range(WIN_PER_CORE):
                rst = sb.tile([128, 257], f32, tag="rst")
                nc.sync.dma_start(out=rst[:], in_=rs_out[nw * 128:(nw + 1) * 128, :])
                den = sb.tile([128, 1], f32, tag="den3")
                nc.vector.tensor_tensor(out=den[:], in0=rst[:, 256:257], in1=eps30[:],
                                        op=mybir.AluOpType.add)
                nc.vector.reciprocal(out=den[:], in_=den[:])
                sagg = sb.tile([128, D], f32, tag="sagg")
                nc.vector.tensor_scalar_mul(out=sagg[:], in0=rst[:, 0:256], scalar1=den[:, :1])
                nc.scalar.dma_start(out=dbg_sagg[nw * 128:(nw + 1) * 128, :], in_=sagg[:])
                for k in range(2):
                    tp = ps.tile([128, 128], f32, tag="ps_tr3")
                    nc.tensor.transpose(out=tp[:], in_=sagg[:, k * 128:(k + 1) * 128], identity=ident[:])
                    nc.vector.tensor_copy(out=sentT[:, k, nw * 128:(nw + 1) * 128], in_=tp[:])
            nodes_t_v = nodes_tc.rearrange("(k p) e -> p k e", p=128)
            for c0 in range(0, CHUNK, 512):
                wid = min(512, CHUNK - c0)
                nT = sb.tile([128, 2, wid], f32r, tag="nT")
                nc.sync.dma_start(out=nT[:], in_=nodes_t_v[:, :, c0:c0 + wid])
                srcs = ([nT[:, k, :] for k in range(2)]
                        + [recvT[:, k, c0:c0 + wid] for k in range(2)]
                        + [sentT[:, k, c0:c0 + wid] for k in range(2)])
                h3 = _mlp(nc, (sb, ps), srcs, nW, nB, 6, wid)
                for s in range(wid // 128):
                    n0 = c0 + s * 128
                    ups = ps.tile([128, D], f32, tag="ps_ups3")
                    for k in range(2):
                        tp2 = ps.tile([128, 128], f32, tag="ps_tr3")
                        nc.tensor.transpose(out=tp2[:], in_=h3[:, k, s * 128:(s + 1) * 128],
                                            identity=ident[:])
                        nc.vector.tensor_copy(out=ups[:, k * 128:(k + 1) * 128], in_=tp2[:])
                    un = _ln_from_psum(nc, sb, ups, ng_bc, nbt_bc, eps_ln)
                    nr = sb.tile([128, D], f32, tag="nr")
                    nc.sync.dma_start(out=nr[:], in_=nodes_rc[n0:n0 + 128, :])
                    on = sb.tile([128, D], f32, tag="on")
                    nc.vector.tensor_tensor(out=on[:], in0=un[:], in1=nr[:], op=mybir.AluOpType.add)
                    nc.scalar.dma_start(out=out_nodes[n0:n0 + 128, :], in_=on[:])

        consts.release()
        dram.release()

    nc.compile()
    return nc


# ----------------------------------------------------------------------------
# entry point
# ----------------------------------------------------------------------------

def kernel(nodes, edges, senders, receivers,
           edge_W0, edge_b0, edge_W1, edge_b1, edge_W2, edge_b2, edge_ln_g, edge_ln_b,
           node_W0, node_b0, node_W1, node_b1, node_W2, node_b2, node_ln_g, node_ln_b,
           recv_attn_w, recv_attn_b, send_attn_w, send_attn_b):
    global LAST_EXEC_NS
    import os
    nodes = np.asarray(nodes, dtype=np.float32)
    edges = np.asarray(edges, dtype=np.float32)
    slot_of_node, nodes_rl, cores, TPW, E_PAD, TPW2, E_PAD2 = _preprocess(
        nodes, edges, senders, receivers)
    nc = build_program(TPW, E_PAD, TPW2, E_PAD2)

    wcat = np.concatenate([np.asarray(recv_attn_w), np.asarray(send_attn_w)], axis=1).astype(np.float32)
    attn_b = np.concatenate([np.asarray(recv_attn_b), np.asarray(send_attn_b)]).astype(np.float32)
    shared = dict(
        nodes_rl=nodes_rl, wcat=wcat, attn_b=attn_b,
        eW0=np.asarray(edge_W0, np.float32), eW1=np.asarray(edge_W1, np.float32),
        eW2=np.asarray(edge_W2, np.float32),
        eb0=np.asarray(edge_b0, np.float32), eb1=np.asarray(edge_b1, np.float32),
        eb2=np.asarray(edge_b2, np.float32),
        eg=np.asarray(edge_ln_g, np.float32), ebt=np.asarray(edge_ln_b, np.float32),
        nW0=np.asarray(node_W0, np.float32), nW1=np.asarray(node_W1, np.float32),
        nW2=np.asarray(node_W2, np.float32),
        nb0=np.asarray(node_b0, np.float32), nb1=np.asarray(node_b1, np.float32),
        nb2=np.asarray(node_b2, np.float32),
        ng=np.asarray(node_ln_g, np.float32), nbt=np.asarray(node_ln_b, np.float32),
    )
    in_maps = []
    for c in range(N_CORES):
        m = dict(shared)
        for k in ("edges_t", "edges_r", "meta", "rslot", "sdest", "p2slot",
                  "nodes_r_chunk", "nodes_t_chunk"):
            tgt = {"nodes_r_chunk": "nodes_rc", "nodes_t_chunk": "nodes_tc"}.get(k, k)
            m[tgt] = cores[c][k]
        in_maps.append(m)

    trace = bool(int(os.environ.get("KERNEL_TRACE", "0")))
    res = run_bass_kernel_spmd(nc, in_maps, list(range(N_CORES)), trace=trace)
    LAST_EXEC_NS = res.exec_time_ns
    global LAST_RESULTS, LAST_PP
    LAST_RESULTS = res.results
    LAST_PP = (slot_of_node, cores)

    edges_out = np.empty((N_EDGES, D), dtype=np.float32)
    nodes_out = np.empty((N_NODES, D), dtype=np.float32)
    inv_slot = np.full(N_SLOTS, -1, dtype=np.int64)
    inv_slot[slot_of_node] = np.arange(N_NODES)
    for c in range(N_CORES):
        eo = res.results[c]["out_edges"]
        ids = cores[c]["edge_ids"]
        real = ids >= 0
        edges_out[ids[real]] = eo[real]
        no = res.results[c]["out_nodes"]
        sl = inv_slot[c * CHUNK:(c + 1) * CHUNK]
        rm = sl >= 0
        nodes_out[sl[rm]] = no[rm]
    return nodes_out, edges_out
